# revision 95
# baseline (speedup 1.0000x reference)
"""Trainium2 Bass kernel for nn_GAT_87617332838818.

Mathematical collapse: the reference GAT aggregates ``alpha * hp[:, dst]``
over incoming edges per destination node.  Since the softmax weights alpha
sum to exactly 1 within each destination segment and the aggregated message
``hp[dst]`` is constant within the segment, the whole message-passing step
is the identity: ``out[n] = hp[n]``.  The network therefore reduces to a
per-node 3-layer MLP:

    logits = W2r @ elu(W1r @ elu(W0r @ x^T))        (per node column)

with W0r = W0.reshape(96,128), W1r = W1.reshape(96,96), W2r = W2.reshape(40,96)
(head-concat order matches the plain reshape).  Verified numerically against
the reference: rel fro err 4e-7 in f32; 4.5e-3 with this device pipeline.

Device strategy (8 NeuronCores, node-sharded 6250 rows each):
  - activations kept feature-on-partition: xT [128, n] fp16, h [96, n] fp16
  - ELU via the split  elu(p') = max(p',0) + (min(exp(p'),1) - 1).
    r = max(p',0) and t' = min(exp(p'),1) - 1 (a dual-op tensor_scalar,
    min-then-add, still 4x) feed TWO accumulating matmuls, so
    W@(r+t') = W@elu(p') IS the next layer's true pre-activation -- no
    bias tensor, bias DMA, bias APs, or host-side correction anywhere
    (bias APs also made the DVE relu a 2-input op, +60ns/pass).  The
    final PSUM is y exactly; out drains are pure copies (scalar.copy on
    ps2 works where scalar.activation+bias crashed the device).
  - column groups sized [128, 512*11, 384, 106]: small first group so the
    drain pipeline starts ~1us earlier (only 128 cols of x gate it), small
    runt last so the end-of-pipeline serial chain is short.  L2 outputs of
    consecutive groups pack vertically (partitions 0:40 / 64:104; PSUM
    base partitions must be 0/32/64) into one [104,512] PSUM tile.
  - steady state is jointly ACT/DVE-bound (~1.55us/512 cols): ACT does the
    2 exps (PSUM 1x, no accel for ACTIVATE) + ~0.75 relu, DVE does the
    rest (relu/min/out-cast).  This is the floor for this dataflow: PSUM
    is readable only by ACT+DVE (gpsimd has no PSUM port, DMA no route),
    TRN2 matmuls write f32 PSUM only (16-bit PSUM is TRN3+), and gpsimd
    fp16 tensor ops measured ~17 cyc/elem (useless).
  - PE HAM clock gate: needs ~5-7us of ~100%-duty PE activity to flip
    1.2->2.4 GHz, and the real MM stream is too gappy to flip it.  Junk
    matmuls fill the DMA-bound head exactly until the first x batch lands,
    then 1-2 junk MMs per early tick keep the stream dense through the
    worst-case flip time.  Without this the whole run stays at 1.2 GHz
    (427ns/MM) and PE gates the pipeline (~+7us).  RESIDUAL VARIANCE: on
    ~1/3 of runs the flip still takes ~8us of near-dense activity (the
    ~100-300ns LDWEIGHTS gaps between MMs dilute the HAM busy fraction),
    costing +2-3us (traced: cold MMs to 15.4us, ACT stalls 2.5us waiting
    on the cold PE).  More junk can't close this without delaying real
    work in good-phase runs; it is the window-phase lottery.
  - software-pipelined emission with 2-tick stage gaps so cross-engine
    dependency latency (exp->min->mm->exp) doesn't leak into the cadence;
    sb pool bufs=6 kills WAR-hazard stalls on drain outputs.
  - stage2 emitted first within each tick so pair drains + DMAs queue
    ahead of that tick's elu work (shortens the tail).  At the very end,
    the last two groups' L1 relus + the final pair drain go to scalar
    while the second-to-last pair drains on DVE — each engine is free
    right when its drain is ready.  Do NOT add more end-of-run DMAs:
    each sync DMA issue costs ~0.8us of serialized queue time, which is
    why the split-last-pair variant regressed.
  - all output DMAs on the sync hwdge queue: gpsimd's software DGE is
    ~2us slower to drain at end-of-program if its DMAs are recent;
    scalar's hwdge queue is not configured by the runtime.
  - x batches [1,2,4,7] groups on sync; w1/w2/bias consts ride gpsimd so
    they never delay the x stream (per-queue DMA bandwidth ~130 GB/s,
    issue cost ~0.6-1us per DMA regardless of size).
  - NOTE: engine passes whose PSUM AP spans two banks crash the device
    (NRT_EXEC_UNIT_UNRECOVERABLE) — keep all PSUM APs within one bank.
  - fp8 x/W0 halves input DMA but costs 3.9e-2 rel err (fails the 2e-2
    gate) — stay fp16 end to end (rel err 9.9e-4).
"""

import os
import sys

import numpy as np

for _p in ("/root/.axon_site/_ro/trn_rl_repo", "/opt/trn_rl_repo"):
    if os.path.isdir(_p) and _p not in sys.path:
        sys.path.append(_p)

import concourse.bass as bass
import concourse.tile as tile
from concourse import bacc, mybir
from concourse.bass_utils import run_bass_kernel_spmd

N_CORES = 8
N_PER = 6250            # 50000 / 8
D_IN = 128
D_HID = 96
D_OUT = 40
MM_N = 512              # matmul moving free-dim (1 PSUM bank)
FDP = 512               # group free-dim (1 PSUM bank)

F16 = mybir.dt.float16
BF16 = mybir.dt.bfloat16
F32 = mybir.dt.float32
F8 = mybir.dt.float8e4

Act = mybir.ActivationFunctionType
Alu = mybir.AluOpType

# group sizes: a small first group so the ACT/DVE drain pipeline starts
# ~1us earlier (only 128 cols of x must arrive first), and a small runt
# last so the end-of-pipeline serial chain (exp->min->mm->drain->dma) is
# short.  sums to N_PER = 6250.
_pairs = [128] + [FDP] * 11 + [384, 106]
assert sum(_pairs) == N_PER
P = len(_pairs)
_pstarts = [sum(_pairs[:i]) for i in range(P)]

# which L0/L1 relu drains go to ACT instead of DVE (by (pair, layer)).
# the last two groups' L1 relus also go to ACT: its exp stream is done by
# then while DVE still has the tail drains.
# measured balance: ACT = 15.2us exp + relus + 0.6 copy, DVE = 10.4 relu
# + 5.4 min + 3.7 cast; 9 relus on ACT equalizes both at ~20.1us
R_DRAIN_ON_ACT = tuple((p, 0) for p in range(P)
                       if p % 4 != 3 and p not in (2, 5, 9, P - 2)) + \
    ((P - 2, 1), (P - 1, 1))
X_BATCHES = [1, 2, 4, 7]
# PE HAM warm-up: the clock gate needs ~5-7us of ~100%-duty PE activity to
# flip 1.2 -> 2.4 GHz, and the real MM stream alone is too gappy early on.
# 3 junk MMs run while the first x batch is in flight (the first real L0
# lands right as its data arrives), then a few more pad the ramp ticks.
N_WARMUP_PRE = 4   # 4th junk fills the ~9.2-9.6us hole before batch-0
                   # data lands, keeping the pre-flip PE stream dense
# ramp junk through tick 7: trimming to {1:2,2:1,3:1} measured ~+2us --
# the late-tick junk guarantees the HAM flip lands during the ramp
# rather than pushing cold-clock time onto real matmuls
WARMUP_TICK = {1: 2, 2: 1, 3: 1, 4: 1, 5: 1, 6: 1, 7: 1, 8: 1, 9: 1}

_batch_of = {}
_b0 = 0
for _bi, _bn in enumerate(X_BATCHES):
    for _g in range(_b0, min(_b0 + _bn, P)):
        _batch_of[_g] = _bi
    _b0 += _bn
assert _b0 >= P


def _mm_splits(fd):
    """Split a pair-tick's fd into <=512 matmul chunks."""
    out = []
    j = 0
    while j < fd:
        out.append((j, min(j + MM_N, fd)))
        j += MM_N
    return out


def _build_program() -> bass.Bass:
    nc = bacc.Bacc(None, target_bir_lowering=False, debug=False)

    # xw packs [w0t | xT]: cols 0..95 = W0^T fp16, cols 96.. = x^T shard
    xw = nc.declare_dram_parameter("xw", [D_IN, D_HID + N_PER], F16,
                                   isOutput=False)
    # wb packs [w1t | w2t] fp16.  No biases exist anywhere: the t pass
    # produces t' = min(e,1)-1, so W@(r+t') IS the next layer's true
    # pre-activation (and the final PSUM is y exactly).
    wb = nc.declare_dram_parameter("wb", [D_HID, D_HID + D_OUT], F16,
                                   isOutput=False)
    # packed output: pair k at cols [512k, 512k+512): rows 0:40 = group 2k,
    # rows 64:104 = group 2k+1 (rows 40:64 unused). Host unpacks.
    yT = nc.declare_dram_parameter("yT", [104, ((P + 1) // 2) * FDP], F16,
                                   isOutput=True)

    st = {}
    st_batch = {}
    batch_tiles = {}

    with tile.TileContext(nc) as tc:
        with (
            tc.tile_pool(name="consts", bufs=1) as consts,
            tc.tile_pool(name="x0", bufs=1) as x0pool,
            tc.tile_pool(name="xin", bufs=3) as xpool,
            tc.tile_pool(name="sb", bufs=8) as sb,
            tc.tile_pool(name="ps0", bufs=3, space="PSUM") as ps0,
            tc.tile_pool(name="ps1", bufs=3, space="PSUM") as ps1,
            tc.tile_pool(name="ps2", bufs=2, space="PSUM") as ps2,
        ):
            # --- PE warm-up junk: one tile doubles as weights + moving
            # operand, one memset on the early-idle vector queue.
            junk = consts.tile([D_IN, MM_N], F8, tag="junk")
            nc.vector.memset(junk[:], 0.0)
            warm = ps2.tile([104, MM_N], F32, tag="p2")

            def warm_mm(n):
                for _ in range(n):
                    nc.tensor.matmul(warm[:D_OUT], junk[:, :D_OUT], junk[:],
                                     start=True, stop=True)

            warm_mm(N_WARMUP_PRE)

            wb_sb = consts.tile([D_HID, D_HID + D_OUT], F16, tag="wb")
            w1_sb = wb_sb[:, :D_HID]
            w2_sb = wb_sb[:, D_HID:D_HID + D_OUT]

            def relu_drain(out_ap, psum_ap, on_act):
                """out = max(psum, 0), PSUM -> SBUF fp16."""
                if on_act:
                    nc.scalar.activation(out_ap, psum_ap, Act.Relu)
                else:
                    nc.vector.tensor_scalar_max(out_ap, psum_ap, 0.0)

            def exp_elu(p, lyr, psum, fd):
                """From psum p': e=exp(p'), r=max(p',0), t'=min(e,1)-1.

                elu(p')+1 = r + (t'+1), so feeding r and t' through the
                accumulating matmuls gives the NEXT layer's true
                pre-activation with no bias anywhere: W@(r+t') = W@elu(p').
                PSUM-reading passes stay within one 512-col bank (engine
                PSUM APs must not cross banks); the SBUF-side t pass runs
                full width."""
                e = sb.tile([D_HID, FDP], F16, tag=f"e{lyr}")
                r = sb.tile([D_HID, FDP], F16, tag=f"r{lyr}")
                for j0, j1 in _mm_splits(fd):
                    nc.scalar.activation(e[:, j0:j1], psum[:, j0:j1],
                                         Act.Exp)
                # relu right after exp (same bank, concurrent ACT+DVE reads
                # cost ~+110ns arbitration on ~half the groups, but moving
                # the relu after the min serializes the per-group chain and
                # measured ~1.9us WORSE -- keep the concurrent form)
                for j0, j1 in _mm_splits(fd):
                    relu_drain(r[:, j0:j1], psum[:, j0:j1],
                               (p, lyr) in R_DRAIN_ON_ACT)
                t = sb.tile([D_HID, FDP], F16, tag=f"t{lyr}")
                # dual-op tensor_scalar (min 1 then add -1), still 4x SBUF.
                # NOTE: gpsimd tensor_scalar_min measured ~7.4us per
                # [96,512] fp16 op (~17 cyc/elem) -- keep mins on DVE (4x)
                nc.vector.tensor_scalar(t[:, :fd], e[:, :fd], 1.0, -1.0,
                                        Alu.min, Alu.add)
                return r, t

            def stage_load(p):
                bi = _batch_of[p]
                if p > 0 and _batch_of[p - 1] == bi:
                    st[p] = st_batch[bi]
                    return
                p1_ = p
                while p1_ + 1 < P and _batch_of[p1_ + 1] == bi:
                    p1_ += 1
                lo = _pstarts[p] + (0 if bi else -D_HID)   # batch 0 incl. w0
                hi = _pstarts[p1_] + _pairs[p1_]
                cols = hi - lo
                pool = x0pool if bi == 0 else xpool
                width = (D_HID + FDP * X_BATCHES[0] if bi == 0
                         else FDP * max(X_BATCHES[1:]))
                xt = pool.tile([D_IN, width], F16,
                               tag=("xt0" if bi == 0 else "xt"))
                # all x batches on sync (xin bufs=3 lets the last batch's
                # transfer start right after the previous one instead of
                # waiting for batch 1's tile to be consumed).  a split
                # sync/gpsimd scheme was tried and regressed ~1.5us: the
                # big gpsimd read contends with sync's transfers exactly
                # when batches 1-2 are needed.
                nc.sync.dma_start(xt[:, :cols], xw[:, D_HID + lo:D_HID + hi])
                st_batch[bi] = {"xt": xt, "base": lo}
                st[p] = st_batch[bi]

            def stage0_mm(p):
                fd = _pairs[p]
                s = dict(st[p])
                st[p] = s
                xo = _pstarts[p] - s["base"]
                w0_sb = batch_tiles["w0"]
                p0 = ps0.tile([D_HID, FDP], F32, tag="p0")
                for j0, j1 in _mm_splits(fd):
                    nc.tensor.matmul(p0[:, j0:j1], w0_sb,
                                     s["xt"][:, xo + j0:xo + j1],
                                     start=True, stop=True)
                s["p0"] = p0

            def stage0_elu(p):
                s = st[p]
                s["r1"], s["t1"] = exp_elu(p, 0, s.pop("p0"), _pairs[p])

            def stage1_mm(p):
                fd = _pairs[p]
                s = st[p]
                p1 = ps1.tile([D_HID, FDP], F32, tag="p1")
                for j0, j1 in _mm_splits(fd):
                    nc.tensor.matmul(p1[:, j0:j1], w1_sb, s["r1"][:, j0:j1],
                                     start=True, stop=False)
                    nc.tensor.matmul(p1[:, j0:j1], w1_sb, s["t1"][:, j0:j1],
                                     start=False, stop=True)
                s["p1"] = p1

            def stage1_elu(p):
                s = st[p]
                s["r2"], s["t2"] = exp_elu(p, 1, s.pop("p1"), _pairs[p])

            pair_state = {}

            def stage2(p):
                fd = _pairs[p]
                s = st.pop(p)
                if p % 2 == 0:
                    p2 = ps2.tile([104, FDP], F32, tag="p2")
                    pair_state[p // 2] = p2
                    rows = slice(0, D_OUT)
                else:
                    p2 = pair_state[p // 2]
                    rows = slice(64, 64 + D_OUT)
                nc.tensor.matmul(p2[rows, :fd], w2_sb, s["r2"][:, :fd],
                                 start=True, stop=False)
                nc.tensor.matmul(p2[rows, :fd], w2_sb, s["t2"][:, :fd],
                                 start=False, stop=True)
                if not ((p % 2 == 1) or (p == P - 1)):
                    return
                nrows = 104 if p % 2 == 1 else D_OUT
                # drain/DMA must cover the wider group of the pair.
                # (splitting the last pair into per-group drains+DMAs was
                # tried and regressed: a third serialized ~0.8us DMA issue
                # on sync costs more than the earlier drain saves)
                wmax = max(fd, _pairs[p - 1]) if p % 2 == 1 else fd
                o = sb.tile([104, FDP], F16, tag="o")
                # pure copy: the -W2@1 bias correction happens on the host.
                # the final pair drains on scalar, the second-to-last on DVE
                # (each engine is free right when its drain becomes ready)
                if p == P - 1:
                    nc.scalar.copy(o[:nrows, :wmax], p2[:nrows, :wmax])
                else:
                    nc.vector.tensor_copy(o[:nrows, :wmax], p2[:nrows, :wmax])
                kp = p // 2
                # all outputs on the sync hwdge queue: gpsimd's software
                # queue is ~2us slower to drain at end-of-program, and
                # scalar's hwdge queue is not configured by the runtime
                nc.sync.dma_start(yT[:, kp * FDP:kp * FDP + wmax],
                                  o[:, :wmax])

            # software-pipelined emission with 2-tick stage gaps: each
            # engine always has a tick of ready work queued, so cross-engine
            # dependency latency (exp -> min -> mm -> exp) doesn't leak into
            # the cadence.  stage2 first within the tick so its pair drain +
            # DMA queue ahead of the tick's elu work (shortens the tail).
            for pp in range(P + 5):
                if pp < P:
                    stage_load(pp)
                    if pp == 0:
                        batch_tiles["w0"] = st[0]["xt"][:, 0:D_HID]
                        # consts ride the gpsimd queue so they don't delay
                        # the x batches on sync
                        nc.gpsimd.dma_start(wb_sb[:], wb[:])
                if 0 <= pp - 5 < P:
                    stage2(pp - 5)
                if 0 <= pp - 1 < P:
                    stage0_mm(pp - 1)
                    stage0_elu(pp - 1)
                if 0 <= pp - 3 < P:
                    stage1_mm(pp - 3)
                    stage1_elu(pp - 3)
                # ramp junk emitted after stage1 so L1 mms aren't queued
                # behind it on the cold PE
                warm_mm(WARMUP_TICK.get(pp, 0))

    nc.compile()
    return nc


_prog_cache = []
last_result = None


def kernel(**inputs) -> np.ndarray:
    global last_result
    x = np.asarray(inputs["x"], np.float32)           # [50000, 128]
    W0 = np.asarray(inputs["W0"], np.float32).reshape(D_HID, D_IN)
    W1 = np.asarray(inputs["W1"], np.float32).reshape(D_HID, D_HID)
    W2 = np.asarray(inputs["W2"], np.float32).reshape(D_OUT, D_HID)

    n = x.shape[0]
    assert n == N_CORES * N_PER, f"unexpected node count {n}"

    xT16 = x.T.astype(np.float16)                            # [128, 50000]
    w0t = W0.T.astype(np.float16)                            # [128, 96]
    w1tb = W1.T.astype(np.float16)                           # [96, 96]
    w2tb = W2.T.astype(np.float16)                           # [96, 40]
    wb = np.ascontiguousarray(
        np.concatenate([w1tb, w2tb], axis=1))                # [96, 136]

    if not _prog_cache:
        _prog_cache.append(_build_program())
    nc = _prog_cache[0]

    in_maps = []
    for i in range(N_CORES):
        xwi = np.ascontiguousarray(
            np.concatenate([w0t, xT16[:, i * N_PER:(i + 1) * N_PER]], axis=1))
        in_maps.append(dict(xw=xwi, wb=wb))
    res = run_bass_kernel_spmd(nc, in_maps, list(range(N_CORES)))
    last_result = res
    out = np.empty((n, D_OUT), np.float32)
    for i in range(N_CORES):
        yt = np.asarray(res.results[i]["yT"], np.float32)  # [104, 3178]
        base = i * N_PER
        for kp in range((P + 1) // 2):
            c0 = kp * FDP
            g0 = 2 * kp
            w0_ = _pairs[g0]
            out[base + _pstarts[g0]:base + _pstarts[g0] + w0_] = \
                yt[0:D_OUT, c0:c0 + w0_].T
            if g0 + 1 < P:
                w1_ = _pairs[g0 + 1]
                out[base + _pstarts[g0 + 1]:base + _pstarts[g0 + 1] + w1_] = \
                    yt[64:64 + D_OUT, c0:c0 + w1_].T
    return out


if __name__ == "__main__":
    data = np.load("/tmp/gat_inputs.npz")
    y = kernel(**{k: data[k] for k in data.files})
    print("out", y.shape, y.dtype, "absmax", np.abs(y).max())



# revision 96
# speedup vs baseline: 1.0145x; 1.0145x over previous
"""Trainium2 Bass kernel for nn_GAT_87617332838818.

Mathematical collapse: the reference GAT aggregates ``alpha * hp[:, dst]``
over incoming edges per destination node.  Since the softmax weights alpha
sum to exactly 1 within each destination segment and the aggregated message
``hp[dst]`` is constant within the segment, the whole message-passing step
is the identity: ``out[n] = hp[n]``.  The network therefore reduces to a
per-node 3-layer MLP:

    logits = W2r @ elu(W1r @ elu(W0r @ x^T))        (per node column)

with W0r = W0.reshape(96,128), W1r = W1.reshape(96,96), W2r = W2.reshape(40,96)
(head-concat order matches the plain reshape).  Verified numerically against
the reference: rel fro err 4e-7 in f32; 4.5e-3 with this device pipeline.

Device strategy (8 NeuronCores, node-sharded 6250 rows each):
  - activations kept feature-on-partition: xT [128, n] fp16, h [96, n] fp16
  - ELU via the split  elu(p') = max(p',0) + (min(exp(p'),1) - 1).
    r = max(p',0) and t' = min(exp(p'),1) - 1 (a dual-op tensor_scalar,
    min-then-add, still 4x) feed TWO accumulating matmuls, so
    W@(r+t') = W@elu(p') IS the next layer's true pre-activation -- no
    bias tensor, bias DMA, bias APs, or host-side correction anywhere
    (bias APs also made the DVE relu a 2-input op, +60ns/pass).  The
    final PSUM is y exactly; out drains are pure copies (scalar.copy on
    ps2 works where scalar.activation+bias crashed the device).
  - column groups sized [128, 512*11, 384, 106]: small first group so the
    drain pipeline starts ~1us earlier (only 128 cols of x gate it), small
    runt last so the end-of-pipeline serial chain is short.  L2 outputs of
    consecutive groups pack vertically (partitions 0:40 / 64:104; PSUM
    base partitions must be 0/32/64) into one [104,512] PSUM tile.
  - steady state is jointly ACT/DVE-bound (~1.55us/512 cols): ACT does the
    2 exps (PSUM 1x, no accel for ACTIVATE) + ~0.75 relu, DVE does the
    rest (relu/min/out-cast).  This is the floor for this dataflow: PSUM
    is readable only by ACT+DVE (gpsimd has no PSUM port, DMA no route),
    TRN2 matmuls write f32 PSUM only (16-bit PSUM is TRN3+), and gpsimd
    fp16 tensor ops measured ~17 cyc/elem (useless).
  - PE HAM clock gate: needs ~5-7us of ~100%-duty PE activity to flip
    1.2->2.4 GHz, and the real MM stream is too gappy to flip it.  Junk
    matmuls fill the DMA-bound head exactly until the first x batch lands,
    then 1-2 junk MMs per early tick keep the stream dense through the
    worst-case flip time.  Without this the whole run stays at 1.2 GHz
    (427ns/MM) and PE gates the pipeline (~+7us).  RESIDUAL VARIANCE: on
    ~1/3 of runs the flip still takes ~8us of near-dense activity (the
    ~100-300ns LDWEIGHTS gaps between MMs dilute the HAM busy fraction),
    costing +2-3us (traced: cold MMs to 15.4us, ACT stalls 2.5us waiting
    on the cold PE).  More junk can't close this without delaying real
    work in good-phase runs; it is the window-phase lottery.
  - software-pipelined emission with 2-tick stage gaps so cross-engine
    dependency latency (exp->min->mm->exp) doesn't leak into the cadence;
    sb pool bufs=6 kills WAR-hazard stalls on drain outputs.
  - stage2 emitted first within each tick so pair drains + DMAs queue
    ahead of that tick's elu work (shortens the tail).  At the very end,
    the last two groups' L1 relus + the final pair drain go to scalar
    while the second-to-last pair drains on DVE — each engine is free
    right when its drain is ready.  Do NOT add more end-of-run DMAs:
    each sync DMA issue costs ~0.8us of serialized queue time, which is
    why the split-last-pair variant regressed.
  - all output DMAs on the sync hwdge queue: gpsimd's software DGE is
    ~2us slower to drain at end-of-program if its DMAs are recent;
    scalar's hwdge queue is not configured by the runtime.
  - x batches [1,2,4,7] groups on sync; w1/w2/bias consts ride gpsimd so
    they never delay the x stream (per-queue DMA bandwidth ~130 GB/s,
    issue cost ~0.6-1us per DMA regardless of size).
  - NOTE: engine passes whose PSUM AP spans two banks crash the device
    (NRT_EXEC_UNIT_UNRECOVERABLE) — keep all PSUM APs within one bank.
  - fp8 x/W0 halves input DMA but costs 3.9e-2 rel err (fails the 2e-2
    gate) — stay fp16 end to end (rel err 9.9e-4).
"""

import os
import sys

import numpy as np

for _p in ("/root/.axon_site/_ro/trn_rl_repo", "/opt/trn_rl_repo"):
    if os.path.isdir(_p) and _p not in sys.path:
        sys.path.append(_p)

import concourse.bass as bass
import concourse.tile as tile
from concourse import bacc, mybir
from concourse.bass_utils import run_bass_kernel_spmd

N_CORES = 8
N_PER = 6250            # 50000 / 8
D_IN = 128
D_HID = 96
D_OUT = 40
MM_N = 512              # matmul moving free-dim (1 PSUM bank)
FDP = 512               # group free-dim (1 PSUM bank)

F16 = mybir.dt.float16
BF16 = mybir.dt.bfloat16
F32 = mybir.dt.float32
F8 = mybir.dt.float8e4

Act = mybir.ActivationFunctionType
Alu = mybir.AluOpType

# group sizes: a small first group so the ACT/DVE drain pipeline starts
# ~1us earlier (only 128 cols of x must arrive first), and a small runt
# last so the end-of-pipeline serial chain (exp->min->mm->drain->dma) is
# short.  sums to N_PER = 6250.
_pairs = [128] + [FDP] * 11 + [384, 106]
assert sum(_pairs) == N_PER
P = len(_pairs)
_pstarts = [sum(_pairs[:i]) for i in range(P)]

# which L0/L1 relu drains go to ACT instead of DVE (by (pair, layer)).
# the last two groups' L1 relus also go to ACT: its exp stream is done by
# then while DVE still has the tail drains.
# measured balance: ACT = 15.2us exp + relus + 0.6 copy, DVE = 10.4 relu
# + 5.4 min + 3.7 cast; 9 relus on ACT equalizes both at ~20.1us
R_DRAIN_ON_ACT = tuple((p, 0) for p in range(P)
                       if p % 4 != 3 and p not in (2, 5, 9, P - 2)) + \
    ((P - 2, 1), (P - 1, 1))
X_BATCHES = [1, 2, 4, 7]
# PE HAM warm-up: the clock gate needs ~5-7us of ~100%-duty PE activity to
# flip 1.2 -> 2.4 GHz, and the real MM stream alone is too gappy early on.
# 3 junk MMs run while the first x batch is in flight (the first real L0
# lands right as its data arrives), then a few more pad the ramp ticks.
N_WARMUP_PRE = 4   # 4th junk fills the ~9.2-9.6us hole before batch-0
                   # data lands, keeping the pre-flip PE stream dense
# ramp junk through tick 7: trimming to {1:2,2:1,3:1} measured ~+2us --
# the late-tick junk guarantees the HAM flip lands during the ramp
# rather than pushing cold-clock time onto real matmuls
WARMUP_TICK = {1: 2, 2: 1, 3: 1, 4: 1, 5: 1, 6: 1, 7: 1, 8: 1, 9: 1}

_batch_of = {}
_b0 = 0
for _bi, _bn in enumerate(X_BATCHES):
    for _g in range(_b0, min(_b0 + _bn, P)):
        _batch_of[_g] = _bi
    _b0 += _bn
assert _b0 >= P


def _mm_splits(fd):
    """Split a pair-tick's fd into <=512 matmul chunks."""
    out = []
    j = 0
    while j < fd:
        out.append((j, min(j + MM_N, fd)))
        j += MM_N
    return out


def _build_program() -> bass.Bass:
    nc = bacc.Bacc(None, target_bir_lowering=False, debug=False)

    # xw packs [w0t | xT]: cols 0..95 = W0^T fp16, cols 96.. = x^T shard
    xw = nc.declare_dram_parameter("xw", [D_IN, D_HID + N_PER], F16,
                                   isOutput=False)
    # wb packs [w1t | w2t] fp16.  No biases exist anywhere: the t pass
    # produces t' = min(e,1)-1, so W@(r+t') IS the next layer's true
    # pre-activation (and the final PSUM is y exactly).
    wb = nc.declare_dram_parameter("wb", [D_HID, D_HID + D_OUT], F16,
                                   isOutput=False)
    # packed output: pair k at cols [512k, 512k+512): rows 0:40 = group 2k,
    # rows 64:104 = group 2k+1 (rows 40:64 unused). Host unpacks.
    yT = nc.declare_dram_parameter("yT", [104, ((P + 1) // 2) * FDP], F16,
                                   isOutput=True)

    st = {}
    st_batch = {}
    batch_tiles = {}

    with tile.TileContext(nc) as tc:
        with (
            tc.tile_pool(name="consts", bufs=1) as consts,
            tc.tile_pool(name="x0", bufs=1) as x0pool,
            tc.tile_pool(name="xin", bufs=3) as xpool,
            tc.tile_pool(name="sb", bufs=6) as sb,
            tc.tile_pool(name="ps0", bufs=3, space="PSUM") as ps0,
            tc.tile_pool(name="ps1", bufs=3, space="PSUM") as ps1,
            tc.tile_pool(name="ps2", bufs=2, space="PSUM") as ps2,
        ):
            # --- PE warm-up junk: one tile doubles as weights + moving
            # operand, one memset on the early-idle vector queue.
            junk = consts.tile([D_IN, MM_N], F8, tag="junk")
            nc.vector.memset(junk[:], 0.0)
            warm = ps2.tile([104, MM_N], F32, tag="p2")

            def warm_mm(n):
                for _ in range(n):
                    nc.tensor.matmul(warm[:D_OUT], junk[:, :D_OUT], junk[:],
                                     start=True, stop=True)

            warm_mm(N_WARMUP_PRE)

            wb_sb = consts.tile([D_HID, D_HID + D_OUT], F16, tag="wb")
            w1_sb = wb_sb[:, :D_HID]
            w2_sb = wb_sb[:, D_HID:D_HID + D_OUT]

            def relu_drain(out_ap, psum_ap, on_act):
                """out = max(psum, 0), PSUM -> SBUF fp16."""
                if on_act:
                    nc.scalar.activation(out_ap, psum_ap, Act.Relu)
                else:
                    nc.vector.tensor_scalar_max(out_ap, psum_ap, 0.0)

            def exp_elu(p, lyr, psum, fd):
                """From psum p': e=exp(p'), r=max(p',0), t'=min(e,1)-1.

                elu(p')+1 = r + (t'+1), so feeding r and t' through the
                accumulating matmuls gives the NEXT layer's true
                pre-activation with no bias anywhere: W@(r+t') = W@elu(p').
                PSUM-reading passes stay within one 512-col bank (engine
                PSUM APs must not cross banks); the SBUF-side t pass runs
                full width."""
                e = sb.tile([D_HID, FDP], F16, tag=f"e{lyr}")
                r = sb.tile([D_HID, FDP], F16, tag=f"r{lyr}")
                for j0, j1 in _mm_splits(fd):
                    nc.scalar.activation(e[:, j0:j1], psum[:, j0:j1],
                                         Act.Exp)
                # relu right after exp (same bank, concurrent ACT+DVE reads
                # cost ~+110ns arbitration on ~half the groups, but moving
                # the relu after the min serializes the per-group chain and
                # measured ~1.9us WORSE -- keep the concurrent form)
                for j0, j1 in _mm_splits(fd):
                    relu_drain(r[:, j0:j1], psum[:, j0:j1],
                               (p, lyr) in R_DRAIN_ON_ACT)
                t = sb.tile([D_HID, FDP], F16, tag=f"t{lyr}")
                # dual-op tensor_scalar (min 1 then add -1), still 4x SBUF.
                # NOTE: gpsimd tensor_scalar_min measured ~7.4us per
                # [96,512] fp16 op (~17 cyc/elem) -- keep mins on DVE (4x)
                nc.vector.tensor_scalar(t[:, :fd], e[:, :fd], 1.0, -1.0,
                                        Alu.min, Alu.add)
                return r, t

            def stage_load(p):
                bi = _batch_of[p]
                if p > 0 and _batch_of[p - 1] == bi:
                    st[p] = st_batch[bi]
                    return
                p1_ = p
                while p1_ + 1 < P and _batch_of[p1_ + 1] == bi:
                    p1_ += 1
                lo = _pstarts[p] + (0 if bi else -D_HID)   # batch 0 incl. w0
                hi = _pstarts[p1_] + _pairs[p1_]
                cols = hi - lo
                pool = x0pool if bi == 0 else xpool
                width = (D_HID + FDP * X_BATCHES[0] if bi == 0
                         else FDP * max(X_BATCHES[1:]))
                xt = pool.tile([D_IN, width], F16,
                               tag=("xt0" if bi == 0 else "xt"))
                # all x batches on sync (xin bufs=3 lets the last batch's
                # transfer start right after the previous one instead of
                # waiting for batch 1's tile to be consumed).  a split
                # sync/gpsimd scheme was tried and regressed ~1.5us: the
                # big gpsimd read contends with sync's transfers exactly
                # when batches 1-2 are needed.
                nc.sync.dma_start(xt[:, :cols], xw[:, D_HID + lo:D_HID + hi])
                st_batch[bi] = {"xt": xt, "base": lo}
                st[p] = st_batch[bi]

            def stage0_mm(p):
                fd = _pairs[p]
                s = dict(st[p])
                st[p] = s
                xo = _pstarts[p] - s["base"]
                w0_sb = batch_tiles["w0"]
                p0 = ps0.tile([D_HID, FDP], F32, tag="p0")
                for j0, j1 in _mm_splits(fd):
                    nc.tensor.matmul(p0[:, j0:j1], w0_sb,
                                     s["xt"][:, xo + j0:xo + j1],
                                     start=True, stop=True)
                s["p0"] = p0

            def stage0_elu(p):
                s = st[p]
                s["r1"], s["t1"] = exp_elu(p, 0, s.pop("p0"), _pairs[p])

            def stage1_mm(p):
                fd = _pairs[p]
                s = st[p]
                p1 = ps1.tile([D_HID, FDP], F32, tag="p1")
                for j0, j1 in _mm_splits(fd):
                    nc.tensor.matmul(p1[:, j0:j1], w1_sb, s["r1"][:, j0:j1],
                                     start=True, stop=False)
                    nc.tensor.matmul(p1[:, j0:j1], w1_sb, s["t1"][:, j0:j1],
                                     start=False, stop=True)
                s["p1"] = p1

            def stage1_elu(p):
                s = st[p]
                s["r2"], s["t2"] = exp_elu(p, 1, s.pop("p1"), _pairs[p])

            pair_state = {}

            def stage2(p):
                fd = _pairs[p]
                s = st.pop(p)
                if p % 2 == 0:
                    p2 = ps2.tile([104, FDP], F32, tag="p2")
                    pair_state[p // 2] = p2
                    rows = slice(0, D_OUT)
                else:
                    p2 = pair_state[p // 2]
                    rows = slice(64, 64 + D_OUT)
                nc.tensor.matmul(p2[rows, :fd], w2_sb, s["r2"][:, :fd],
                                 start=True, stop=False)
                nc.tensor.matmul(p2[rows, :fd], w2_sb, s["t2"][:, :fd],
                                 start=False, stop=True)
                if not ((p % 2 == 1) or (p == P - 1)):
                    return
                nrows = 104 if p % 2 == 1 else D_OUT
                # drain/DMA must cover the wider group of the pair.
                # (splitting the last pair into per-group drains+DMAs was
                # tried and regressed: a third serialized ~0.8us DMA issue
                # on sync costs more than the earlier drain saves)
                wmax = max(fd, _pairs[p - 1]) if p % 2 == 1 else fd
                o = sb.tile([104, FDP], F16, tag="o")
                # pure copy: the -W2@1 bias correction happens on the host.
                # the final pair drains on scalar, the second-to-last on DVE
                # (each engine is free right when its drain becomes ready)
                if p == P - 1:
                    nc.scalar.copy(o[:nrows, :wmax], p2[:nrows, :wmax])
                else:
                    nc.vector.tensor_copy(o[:nrows, :wmax], p2[:nrows, :wmax])
                kp = p // 2
                # all outputs on the sync hwdge queue: gpsimd's software
                # queue is ~2us slower to drain at end-of-program, and
                # scalar's hwdge queue is not configured by the runtime
                nc.sync.dma_start(yT[:, kp * FDP:kp * FDP + wmax],
                                  o[:, :wmax])

            # software-pipelined emission with 2-tick stage gaps: each
            # engine always has a tick of ready work queued, so cross-engine
            # dependency latency (exp -> min -> mm -> exp) doesn't leak into
            # the cadence.  stage2 first within the tick so its pair drain +
            # DMA queue ahead of the tick's elu work (shortens the tail).
            for pp in range(P + 5):
                if pp < P:
                    stage_load(pp)
                    if pp == 0:
                        batch_tiles["w0"] = st[0]["xt"][:, 0:D_HID]
                        # consts ride the gpsimd queue so they don't delay
                        # the x batches on sync
                        nc.gpsimd.dma_start(wb_sb[:], wb[:])
                if 0 <= pp - 5 < P:
                    stage2(pp - 5)
                if 0 <= pp - 1 < P:
                    stage0_mm(pp - 1)
                    stage0_elu(pp - 1)
                if 0 <= pp - 3 < P:
                    stage1_mm(pp - 3)
                    stage1_elu(pp - 3)
                # ramp junk emitted after stage1 so L1 mms aren't queued
                # behind it on the cold PE
                warm_mm(WARMUP_TICK.get(pp, 0))

    nc.compile()
    return nc


_prog_cache = []
last_result = None


def kernel(**inputs) -> np.ndarray:
    global last_result
    x = np.asarray(inputs["x"], np.float32)           # [50000, 128]
    W0 = np.asarray(inputs["W0"], np.float32).reshape(D_HID, D_IN)
    W1 = np.asarray(inputs["W1"], np.float32).reshape(D_HID, D_HID)
    W2 = np.asarray(inputs["W2"], np.float32).reshape(D_OUT, D_HID)

    n = x.shape[0]
    assert n == N_CORES * N_PER, f"unexpected node count {n}"

    xT16 = x.T.astype(np.float16)                            # [128, 50000]
    w0t = W0.T.astype(np.float16)                            # [128, 96]
    w1tb = W1.T.astype(np.float16)                           # [96, 96]
    w2tb = W2.T.astype(np.float16)                           # [96, 40]
    wb = np.ascontiguousarray(
        np.concatenate([w1tb, w2tb], axis=1))                # [96, 136]

    if not _prog_cache:
        _prog_cache.append(_build_program())
    nc = _prog_cache[0]

    in_maps = []
    for i in range(N_CORES):
        xwi = np.ascontiguousarray(
            np.concatenate([w0t, xT16[:, i * N_PER:(i + 1) * N_PER]], axis=1))
        in_maps.append(dict(xw=xwi, wb=wb))
    res = run_bass_kernel_spmd(nc, in_maps, list(range(N_CORES)))
    last_result = res
    out = np.empty((n, D_OUT), np.float32)
    for i in range(N_CORES):
        yt = np.asarray(res.results[i]["yT"], np.float32)  # [104, 3178]
        base = i * N_PER
        for kp in range((P + 1) // 2):
            c0 = kp * FDP
            g0 = 2 * kp
            w0_ = _pairs[g0]
            out[base + _pstarts[g0]:base + _pstarts[g0] + w0_] = \
                yt[0:D_OUT, c0:c0 + w0_].T
            if g0 + 1 < P:
                w1_ = _pairs[g0 + 1]
                out[base + _pstarts[g0 + 1]:base + _pstarts[g0 + 1] + w1_] = \
                    yt[64:64 + D_OUT, c0:c0 + w1_].T
    return out


if __name__ == "__main__":
    data = np.load("/tmp/gat_inputs.npz")
    y = kernel(**{k: data[k] for k in data.files})
    print("out", y.shape, y.dtype, "absmax", np.abs(y).max())

